# revision 8
# baseline (speedup 1.0000x reference)
"""Dense dot-product attention on 8 Trainium2 NeuronCores.

Problem: query/key/value [32, 2048, 64] fp32 -> softmax(Q K^T / 8) V.
Sharding: batch dim split 4-per-core across 8 cores (data parallel, no
collectives). Each core computes full attention for its 4 batches.

Design (ScalarE-exp-bound, PE kept below the exp floor):
  - All matmuls fp16 (1 cycle/col on the PE; keeps HAM at 2.4 GHz).
  - Q/K transposed to [d, seq] layout by the DMA xbar transpose engine
    (fp16, hidden under compute) -- zero PE transposes. K lands
    pair-packed: khT2[:, p, :] holds K^T tile 2p in partitions 0-63 and
    tile 2p+1 in partitions 64-127, feeding the two 64-row S-matmul
    strips directly. Q is reshuffled into a flat [64, 2048] layout and
    duplicated into both partition halves.
  - S^T[k, q] accumulates into a single persistent PSUM tile
    [128, 3072] = three rotating 1024-col k-tile buffers. exp runs on
    ScalarE over two adjacent buffers at once (N=2048) whenever the
    rotation allows: ~75% of columns go through N=2048 activations,
    cutting the fixed ~350-cycle per-instruction overhead.
  - PV uses the stationary-streaming swap: the exp'd score chunk
    [128k, 128q] is the weight (LoadStationary, FWL at fp16), and
    [V | ones] [128k, 65] streams through. Output accumulates q-MAJOR
    [128q, 65] in PSUM: no output transpose at all, and column 64 is
    the softmax denominator for free. Weight loads double-buffer
    against the streaming matmuls, sustaining ~55-70 ns per 128x128
    weight tile.
  - Normalize: reciprocal of column 64 + per-q-block scalar multiply on
    the DVE, then DMA out.
  - Software pipeline: each plan slot issues S-matmuls for tile group
    i, the exp for group i, then PV matmuls for group i-1, so the PE
    streams while ScalarE exps and vice versa.
"""

import numpy as np

B, L, D = 32, 2048, 64
NCORES = 8
B_SH = B // NCORES          # 4 batches per core
LT = L // 128               # 16 k-tiles of 128
NQH = 2                     # q processed in halves of 1024
QHW = L // NQH              # 1024
NBLK = QHW // 128           # 8 q-blocks of 128 per qh
SCALE = 1.0 / np.sqrt(np.float32(D))  # 0.125

_cached = {}


def _plan_qh(phase):
    """Greedy exp-tile plan for one qh: list of ('pair'|'single', t).

    Score buffer of k-tile t is (phase + t) % 3 (1024 cols each inside
    the [128, 3072] PSUM tile). A pair needs buffers (0,1) or (1,2)
    (contiguous columns) -> exp N=2048; otherwise single N=1024.
    """
    plans = []
    t = 0
    while t < LT:
        b = (phase + t) % 3
        if b <= 1 and t + 1 < LT:
            plans.append(("pair", t))
            t += 2
        else:
            plans.append(("single", t))
            t += 1
    return plans


def _build():
    import concourse.bacc as bacc
    import concourse.tile as tile
    from concourse import mybir

    f32 = mybir.dt.float32
    fp16 = mybir.dt.float16
    Exp = mybir.ActivationFunctionType.Exp

    nc = bacc.Bacc("TRN2", target_bir_lowering=False, debug=False)

    q_d = nc.dram_tensor("query", [B_SH, L, D], f32, kind="ExternalInput")
    k_d = nc.dram_tensor("key", [B_SH, L, D], f32, kind="ExternalInput")
    v_d = nc.dram_tensor("value", [B_SH, L, D], f32, kind="ExternalInput")
    o_d = nc.dram_tensor("out", [B_SH, L, D], f32, kind="ExternalOutput")

    with tile.TileContext(nc) as tc:
        with (
            tc.tile_pool(name="consts", bufs=1) as consts,
            tc.tile_pool(name="nat", bufs=2) as nat,
            tc.tile_pool(name="nath", bufs=2) as nath,
            tc.tile_pool(name="vst", bufs=2) as vst,
            tc.tile_pool(name="qkt", bufs=2) as qkt,
            tc.tile_pool(name="vr", bufs=2) as vrp,
            tc.tile_pool(name="er2", bufs=2) as er2p,
            tc.tile_pool(name="er1", bufs=2) as er1p,
            tc.tile_pool(name="osb", bufs=2) as osbp,
            tc.tile_pool(name="of32", bufs=2) as of32p,
            tc.tile_pool(name="rz", bufs=8) as rzp,
            tc.tile_pool(name="sps", bufs=1, space="PSUM") as spsq,
            tc.tile_pool(name="pvps", bufs=1, space="PSUM") as pvps,
        ):
            # ACT table load + warmers first: they hide under the
            # initial DMA latency and push HAM to 2.4 GHz.
            wsrc = consts.tile([128, 512], fp16)
            nc.vector.memset(wsrc, 1.0)
            dummy = consts.tile([128, 1], f32)
            nc.vector.memset(dummy, 0.0)
            nc.scalar.activation(out=dummy, in_=dummy, func=Exp, scale=1.0)

            S3 = spsq.tile([128, 3 * QHW], f32, tag="S3")

            def warmer(n=512):
                nc.tensor.matmul(S3[0:64, 2 * QHW:2 * QHW + n], wsrc[:, 0:64],
                                 wsrc[:, 0:n], start=True, stop=True,
                                 skip_group_check=True)

            # per-batch persistent tiles
            tiles = {}  # b -> (qhT, khT2, vr)

            def prep_load(b):
                """DMA loads + fp16 casts + xbar transposes for batch b.

                Returns a list of thunks (woven into the previous
                batch's stream). Dependencies are tracked by Tile, so
                placement only affects issue order.
                """
                q_nat = nat.tile([128, LT, D], f32, tag="qnat")
                k_nat = nat.tile([128, LT, D], f32, tag="knat")
                qh_nat = nath.tile([128, LT, D], fp16, tag="qh_nat")
                kh_nat = nath.tile([128, LT, D], fp16, tag="kh_nat")
                qhT = qkt.tile([128, L], fp16, tag="qhT")
                khT2 = qkt.tile([128, LT // 2, 128], fp16, tag="khT2")
                qstage = qkt.tile([128, LT // 2, 128], fp16, tag="qstage")
                v_stage = vst.tile([128, LT, D], f32, tag="vstage")
                vr = vrp.tile([128, LT, D + 1], fp16, tag="vr")

                q_r = q_d.ap()[b].rearrange("(t p) d -> p t d", p=128)
                k_r = k_d.ap()[b].rearrange("(t p) d -> p t d", p=128)

                jobs = []

                def j(fn):
                    jobs.append(fn)

                # loads (k first: S-matmul weights come from K)
                j(lambda: nc.sync.dma_start(out=k_nat[:, 0:4, :], in_=k_r[:, 0:4, :]))
                j(lambda: nc.sync.dma_start(out=q_nat[:, 0:8, :], in_=q_r[:, 0:8, :]))
                j(lambda: nc.sync.dma_start(out=k_nat[:, 4:LT, :], in_=k_r[:, 4:LT, :]))
                j(lambda: nc.sync.dma_start(out=q_nat[:, 8:LT, :], in_=q_r[:, 8:LT, :]))
                j(lambda: nc.sync.dma_start(
                    out=v_stage, in_=v_d.ap()[b].rearrange("(t p) d -> p t d", p=128)))
                # casts
                j(lambda: nc.vector.tensor_copy(out=kh_nat[:, 0:4, :], in_=k_nat[:, 0:4, :]))
                j(lambda: nc.vector.tensor_copy(out=qh_nat[:, 0:8, :], in_=q_nat[:, 0:8, :]))
                j(lambda: nc.vector.tensor_copy(out=kh_nat[:, 4:LT, :], in_=k_nat[:, 4:LT, :]))
                j(lambda: nc.vector.tensor_copy(out=qh_nat[:, 8:LT, :], in_=q_nat[:, 8:LT, :]))
                # xbar pair-transposes: [128, 2, 64] -> [128, 128] with
                # tile 2p in partitions 0-63, tile 2p+1 in 64-127.
                for p in range(LT // 2):
                    def ktr(p=p):
                        nc.sync.dma_start_transpose(
                            out=khT2[:, p, :], in_=kh_nat[:, 2 * p:2 * p + 2, :])
                    j(ktr)
                for p in range(LT // 2):
                    def qtr(p=p):
                        nc.sync.dma_start_transpose(
                            out=qstage[:, p, :], in_=qh_nat[:, 2 * p:2 * p + 2, :])
                    j(qtr)
                # reshuffle Q^T pairs into flat [64, 2048], then dup to
                # the upper partition half for the strip-b matmuls.
                qhT_t = qhT[0:64, :].rearrange("p (t e c) -> p t e c", e=2, c=128)

                def reshuf_even():
                    nc.sync.dma_start(out=qhT_t[:, :, 0, :], in_=qstage[0:64, :, :])

                def reshuf_odd():
                    nc.sync.dma_start(out=qhT_t[:, :, 1, :], in_=qstage[64:128, :, :])

                j(reshuf_even)
                j(reshuf_odd)
                j(lambda: nc.sync.dma_start(out=qhT[64:128, :], in_=qhT[0:64, :]))
                # V: cast into [V | ones]
                j(lambda: nc.vector.tensor_copy(out=vr[:, :, 0:D], in_=v_stage))
                j(lambda: nc.vector.memset(vr[:, :, D:D + 1], 1.0))

                tiles[b] = (qhT, khT2, vr)
                return jobs

            pending = []   # deferred qh-finish jobs woven into later slots

            state = {"g": 0}  # global k-tile counter (phase for buffers)

            def main(b, next_jobs):
                qhT, khT2, vr = tiles.pop(b)
                slot = 0

                def weave(n):
                    nonlocal slot
                    if pending:
                        pending.pop(0)()
                    for _ in range(n):
                        if slot < len(next_jobs):
                            next_jobs[slot]()
                            slot += 1

                def s_mms(t, q0):
                    buf = (state["g"] + t) % 3
                    h = slice(0, 64) if t % 2 == 0 else slice(64, 128)
                    w = khT2[h, t // 2, :]
                    for jj in range(QHW // 512):
                        nc.tensor.matmul(
                            S3[:, buf * QHW + jj * 512: buf * QHW + (jj + 1) * 512],
                            w, qhT[h, q0 + jj * 512: q0 + (jj + 1) * 512],
                            start=True, stop=True)

                def pv_mms(pv, e_of_t):
                    # start=True clears the whole PSUM *bank*, so only
                    # the first block touching each bank (4 blocks/bank
                    # at 512B stride) may carry it; the other blocks'
                    # first write relies on has_written=0 -> overwrite.
                    for t, e_chunk in e_of_t:
                        for blk in range(NBLK):
                            nc.tensor.matmul(
                                pv[:, blk, 0:D + 1],
                                e_chunk[:, blk * 128:(blk + 1) * 128],
                                vr[:, t, :],
                                start=(t == 0 and blk % 4 == 0),
                                stop=(t == LT - 1),
                                skip_group_check=True)

                for qh in range(NQH):
                    q0 = qh * QHW
                    plans = _plan_qh(state["g"] % 3)
                    # padded to 128 f32 per block so every matmul's
                    # 260B output region stays inside one PSUM bank
                    pv = pvps.tile([128, NBLK, 128], f32, tag="pv")
                    prev_pv = None   # (pv, e_of_t) awaiting issue

                    for kind, t in plans:
                        weave(2)
                        # S matmuls for this group
                        s_mms(t, q0)
                        if kind == "pair":
                            s_mms(t + 1, q0)
                        # exp on ScalarE
                        buf = (state["g"] + t) % 3
                        if kind == "pair":
                            e = er2p.tile([128, 2 * QHW], fp16, tag="e2")
                            nc.scalar.activation(
                                out=e, in_=S3[:, buf * QHW:(buf + 2) * QHW],
                                func=Exp, scale=float(SCALE))
                            group = [(t, e[:, 0:QHW]), (t + 1, e[:, QHW:2 * QHW])]
                        else:
                            e = er1p.tile([128, QHW], fp16, tag="e1")
                            nc.scalar.activation(
                                out=e, in_=S3[:, buf * QHW:(buf + 1) * QHW],
                                func=Exp, scale=float(SCALE))
                            group = [(t, e)]
                        # PV for the previous group (PE streams under
                        # the current group's exp)
                        if prev_pv is not None:
                            pv_mms(*prev_pv)
                        prev_pv = (pv, group)

                    # last group of the qh
                    pv_mms(*prev_pv)
                    state["g"] += LT

                    # deferred finish: evacuate q-major [128q, 8, 65],
                    # normalize by column 64, store.
                    o_sb = osbp.tile([128, NBLK, D + 1], f32, tag="osb")
                    o_f = of32p.tile([128, NBLK, D], f32, tag="of")

                    def evac_job(pv=pv, o_sb=o_sb):
                        nc.vector.tensor_copy(out=o_sb, in_=pv[:, :, 0:D + 1])
                    pending.append(evac_job)

                    for blk in range(NBLK):
                        def norm_job(blk=blk, o_sb=o_sb, o_f=o_f):
                            rz = rzp.tile([128, 1], f32, tag="rz")
                            nc.vector.reciprocal(
                                out=rz, in_=o_sb[:, blk, D:D + 1])
                            nc.vector.tensor_scalar_mul(
                                out=o_f[:, blk, :], in0=o_sb[:, blk, 0:D],
                                scalar1=rz)
                        pending.append(norm_job)

                    def store_job(b=b, q0=q0, o_f=o_f):
                        nc.sync.dma_start(
                            out=o_d.ap()[b, q0:q0 + QHW, :].rearrange(
                                "(t p) d -> p t d", p=128),
                            in_=o_f)
                    pending.append(store_job)

                while slot < len(next_jobs):
                    next_jobs[slot]()
                    slot += 1

            for _ in range(12):
                warmer()
            jobs0 = prep_load(0)
            for job in jobs0:
                job()
            for b in range(B_SH):
                nxt = prep_load(b + 1) if b + 1 < B_SH else []
                main(b, nxt)
            for job in pending:
                job()

    nc.finalize()
    return nc


def _get_nc():
    if "nc" not in _cached:
        _cached["nc"] = _build()
    return _cached["nc"]


def kernel(query, key, value):
    from concourse.bass_utils import run_bass_kernel_spmd

    nc = _get_nc()
    query = np.ascontiguousarray(query, dtype=np.float32)
    key = np.ascontiguousarray(key, dtype=np.float32)
    value = np.ascontiguousarray(value, dtype=np.float32)

    in_maps = []
    for c in range(NCORES):
        sl = slice(c * B_SH, (c + 1) * B_SH)
        in_maps.append({
            "query": query[sl], "key": key[sl], "value": value[sl]})

    res = run_bass_kernel_spmd(nc, in_maps, core_ids=list(range(NCORES)))
    out = np.concatenate([r["out"] for r in res.results], axis=0)
    return out


# revision 9
# speedup vs baseline: 1.0182x; 1.0182x over previous
"""Dense dot-product attention on 8 Trainium2 NeuronCores.

Problem: query/key/value [32, 2048, 64] fp32 -> softmax(Q K^T / 8) V.
Sharding: batch dim split 4-per-core across 8 cores (data parallel, no
collectives). Each core computes full attention for its 4 batches.

Design (ScalarE-exp-bound, PE kept below the exp floor):
  - All matmuls fp16 (1 cycle/col on the PE; keeps HAM at 2.4 GHz).
  - Q/K transposed to [d, seq] layout by the DMA xbar transpose engine
    (fp16, hidden under compute) -- zero PE transposes. K lands
    pair-packed: khT2[:, p, :] holds K^T tile 2p in partitions 0-63 and
    tile 2p+1 in partitions 64-127, feeding the two 64-row S-matmul
    strips directly. Q is reshuffled into a flat [64, 2048] layout and
    duplicated into both partition halves.
  - S^T[k, q] accumulates into a single persistent PSUM tile
    [128, 3072] = three rotating 1024-col k-tile buffers. exp runs on
    ScalarE over two adjacent buffers at once (N=2048) whenever the
    rotation allows: ~75% of columns go through N=2048 activations,
    cutting the fixed ~350-cycle per-instruction overhead.
  - PV uses the stationary-streaming swap: the exp'd score chunk
    [128k, 128q] is the weight (LoadStationary, FWL at fp16), and
    [V | ones] [128k, 65] streams through. Output accumulates q-MAJOR
    [128q, 65] in PSUM: no output transpose at all, and column 64 is
    the softmax denominator for free. Weight loads double-buffer
    against the streaming matmuls, sustaining ~55-70 ns per 128x128
    weight tile.
  - Normalize: reciprocal of column 64 + per-q-block scalar multiply on
    the DVE, then DMA out.
  - Software pipeline: each plan slot issues S-matmuls for tile group
    i, the exp for group i, then PV matmuls for group i-1, so the PE
    streams while ScalarE exps and vice versa.
"""

import numpy as np

B, L, D = 32, 2048, 64
NCORES = 8
B_SH = B // NCORES          # 4 batches per core
LT = L // 128               # 16 k-tiles of 128
NQH = 4                     # q processed in chunks of 512
QHW = L // NQH              # 512
NBLK = QHW // 128           # 4 q-blocks of 128 per qh
SCALE = 1.0 / np.sqrt(np.float32(D))  # 0.125

_cached = {}


_NTILES = {"tri": 3, "pair": 2, "single": 1}


def _plan_qh(phase):
    """Greedy exp-tile plan for one qh: list of (kind, t).

    Score buffer of k-tile t is (phase + t) % 6, one PSUM bank
    ([128, 512] f32) each inside the [128, 3072] tile. Groups need
    contiguous buffers (no wraparound): triples from buf<=3, pairs
    from buf<=4. The 6-deep rotation keeps the S-write >= 2 plans
    behind the exp that last read the buffer, so nothing stalls.
    """
    plans = []
    t = 0
    while t < LT:
        b = (phase + t) % 6
        rem = LT - t
        if b <= 3 and rem >= 3:
            plans.append(("tri", t))
            t += 3
        elif b <= 4 and rem >= 2:
            plans.append(("pair", t))
            t += 2
        else:
            plans.append(("single", t))
            t += 1
    return plans


def _build():
    import concourse.bacc as bacc
    import concourse.tile as tile
    from concourse import mybir

    f32 = mybir.dt.float32
    fp16 = mybir.dt.float16
    Exp = mybir.ActivationFunctionType.Exp

    nc = bacc.Bacc("TRN2", target_bir_lowering=False, debug=False)

    q_d = nc.dram_tensor("query", [B_SH, L, D], f32, kind="ExternalInput")
    k_d = nc.dram_tensor("key", [B_SH, L, D], f32, kind="ExternalInput")
    v_d = nc.dram_tensor("value", [B_SH, L, D], f32, kind="ExternalInput")
    o_d = nc.dram_tensor("out", [B_SH, L, D], f32, kind="ExternalOutput")

    with tile.TileContext(nc) as tc:
        with (
            tc.tile_pool(name="consts", bufs=1) as consts,
            tc.tile_pool(name="nat", bufs=2) as nat,
            tc.tile_pool(name="nath", bufs=2) as nath,
            tc.tile_pool(name="vst", bufs=2) as vst,
            tc.tile_pool(name="qkt", bufs=2) as qkt,
            tc.tile_pool(name="vr", bufs=2) as vrp,
            tc.tile_pool(name="er", bufs=2) as erp,
            tc.tile_pool(name="of32", bufs=2) as of32p,
            tc.tile_pool(name="rz", bufs=8) as rzp,
            tc.tile_pool(name="sps", bufs=1, space="PSUM") as spsq,
            tc.tile_pool(name="pvps", bufs=2, space="PSUM") as pvps,
        ):
            # ACT table load + warmers first: they hide under the
            # initial DMA latency and push HAM to 2.4 GHz.
            wsrc = consts.tile([128, 512], fp16)
            nc.vector.memset(wsrc, 1.0)
            dummy = consts.tile([128, 1], f32)
            nc.vector.memset(dummy, 0.0)
            nc.scalar.activation(out=dummy, in_=dummy, func=Exp, scale=1.0)

            S6 = spsq.tile([128, 6 * QHW], f32, tag="S6")

            def warmer(n=512):
                nc.tensor.matmul(S6[0:64, 0:n], wsrc[:, 0:64],
                                 wsrc[:, 0:n], start=True, stop=True,
                                 skip_group_check=True)

            # per-batch persistent tiles
            tiles = {}  # b -> (qhT, khT2, vr)

            def prep_load(b):
                """DMA loads + fp16 casts + xbar transposes for batch b.

                Returns a list of thunks (woven into the previous
                batch's stream). Dependencies are tracked by Tile, so
                placement only affects issue order.
                """
                q_nat = nat.tile([128, LT, D], f32, tag="qnat")
                k_nat = nat.tile([128, LT, D], f32, tag="knat")
                qh_nat = nath.tile([128, LT, D], fp16, tag="qh_nat")
                kh_nat = nath.tile([128, LT, D], fp16, tag="kh_nat")
                qhT = qkt.tile([128, L], fp16, tag="qhT")
                khT2 = qkt.tile([128, LT // 2, 128], fp16, tag="khT2")
                qstage = qkt.tile([128, LT // 2, 128], fp16, tag="qstage")
                v_stage = vst.tile([128, LT, D], f32, tag="vstage")
                vr = vrp.tile([128, LT, D + 1], fp16, tag="vr")

                q_r = q_d.ap()[b].rearrange("(t p) d -> p t d", p=128)
                k_r = k_d.ap()[b].rearrange("(t p) d -> p t d", p=128)

                jobs = []

                def j(fn):
                    jobs.append(fn)

                # loads (k first: S-matmul weights come from K)
                j(lambda: nc.sync.dma_start(out=k_nat[:, 0:4, :], in_=k_r[:, 0:4, :]))
                j(lambda: nc.sync.dma_start(out=q_nat[:, 0:8, :], in_=q_r[:, 0:8, :]))
                j(lambda: nc.sync.dma_start(out=k_nat[:, 4:LT, :], in_=k_r[:, 4:LT, :]))
                j(lambda: nc.sync.dma_start(out=q_nat[:, 8:LT, :], in_=q_r[:, 8:LT, :]))
                j(lambda: nc.sync.dma_start(
                    out=v_stage, in_=v_d.ap()[b].rearrange("(t p) d -> p t d", p=128)))
                # casts
                j(lambda: nc.vector.tensor_copy(out=kh_nat[:, 0:4, :], in_=k_nat[:, 0:4, :]))
                j(lambda: nc.vector.tensor_copy(out=qh_nat[:, 0:8, :], in_=q_nat[:, 0:8, :]))
                j(lambda: nc.vector.tensor_copy(out=kh_nat[:, 4:LT, :], in_=k_nat[:, 4:LT, :]))
                j(lambda: nc.vector.tensor_copy(out=qh_nat[:, 8:LT, :], in_=q_nat[:, 8:LT, :]))
                # xbar pair-transposes: [128, 2, 64] -> [128, 128] with
                # tile 2p in partitions 0-63, tile 2p+1 in 64-127.
                for p in range(LT // 2):
                    def ktr(p=p):
                        nc.sync.dma_start_transpose(
                            out=khT2[:, p, :], in_=kh_nat[:, 2 * p:2 * p + 2, :])
                    j(ktr)
                for p in range(LT // 2):
                    def qtr(p=p):
                        nc.sync.dma_start_transpose(
                            out=qstage[:, p, :], in_=qh_nat[:, 2 * p:2 * p + 2, :])
                    j(qtr)
                # reshuffle Q^T pairs into flat [64, 2048], then dup to
                # the upper partition half for the strip-b matmuls.
                qhT_t = qhT[0:64, :].rearrange("p (t e c) -> p t e c", e=2, c=128)

                def reshuf_even():
                    nc.sync.dma_start(out=qhT_t[:, :, 0, :], in_=qstage[0:64, :, :])

                def reshuf_odd():
                    nc.sync.dma_start(out=qhT_t[:, :, 1, :], in_=qstage[64:128, :, :])

                j(reshuf_even)
                j(reshuf_odd)
                j(lambda: nc.sync.dma_start(out=qhT[64:128, :], in_=qhT[0:64, :]))
                # V: cast into [V | ones]
                j(lambda: nc.vector.tensor_copy(out=vr[:, :, 0:D], in_=v_stage))
                j(lambda: nc.vector.memset(vr[:, :, D:D + 1], 1.0))

                tiles[b] = (qhT, khT2, vr)
                return jobs

            pending = []   # deferred qh-finish jobs woven into later slots

            state = {"g": 0}  # global k-tile counter (phase for buffers)

            def main(b, next_jobs):
                qhT, khT2, vr = tiles.pop(b)
                slot = 0

                def weave(n):
                    nonlocal slot
                    if pending:
                        pending.pop(0)()
                    for _ in range(n):
                        if slot < len(next_jobs):
                            next_jobs[slot]()
                            slot += 1

                def s_mms(t, q0):
                    buf = (state["g"] + t) % 6
                    h = slice(0, 64) if t % 2 == 0 else slice(64, 128)
                    w = khT2[h, t // 2, :]
                    nc.tensor.matmul(
                        S6[:, buf * QHW:(buf + 1) * QHW],
                        w, qhT[h, q0:q0 + QHW],
                        start=True, stop=True)

                def pv_mms(pv, e_of_t):
                    # start=True clears the whole PSUM *bank*, so only
                    # the first block touching each bank (4 blocks/bank
                    # at 512B stride) may carry it; the other blocks'
                    # first write relies on has_written=0 -> overwrite.
                    for t, e_chunk in e_of_t:
                        for blk in range(NBLK):
                            nc.tensor.matmul(
                                pv[:, blk, 0:D + 1],
                                e_chunk[:, blk * 128:(blk + 1) * 128],
                                vr[:, t, :],
                                start=(t == 0 and blk % 4 == 0),
                                stop=(t == LT - 1),
                                skip_group_check=True)

                for qh in range(NQH):
                    q0 = qh * QHW
                    plans = _plan_qh(state["g"] % 6)
                    # padded to 128 f32 per block so every matmul's
                    # 260B output region stays inside one PSUM bank
                    pv = pvps.tile([128, NBLK, 128], f32, tag="pv")
                    prev_pv = None   # (pv, e_of_t) awaiting issue

                    for kind, t in plans:
                        weave(2)
                        n = _NTILES[kind]
                        for i in range(n):
                            s_mms(t + i, q0)
                        buf = (state["g"] + t) % 6
                        e = erp.tile([128, 3 * QHW], fp16, tag="e")
                        nc.scalar.activation(
                            out=e[:, 0:n * QHW],
                            in_=S6[:, buf * QHW:(buf + n) * QHW],
                            func=Exp, scale=float(SCALE))
                        group = [(t + i, e[:, i * QHW:(i + 1) * QHW])
                                 for i in range(n)]
                        # PV for the previous group (PE streams under
                        # the current group's exp)
                        if prev_pv is not None:
                            pv_mms(*prev_pv)
                        prev_pv = (pv, group)

                    # last group of the qh
                    pv_mms(*prev_pv)
                    state["g"] += LT

                    # deferred finish: normalize q-major [128q, blk, 65]
                    # straight out of PSUM (recip of col 64, then scale)
                    o_f = of32p.tile([128, NBLK, D], f32, tag="of")

                    for blk in range(NBLK):
                        def norm_job(blk=blk, pv=pv, o_f=o_f):
                            rz = rzp.tile([128, 1], f32, tag="rz")
                            nc.vector.reciprocal(
                                out=rz, in_=pv[:, blk, D:D + 1])
                            nc.vector.tensor_scalar_mul(
                                out=o_f[:, blk, :], in0=pv[:, blk, 0:D],
                                scalar1=rz)
                        pending.append(norm_job)

                    def store_job(b=b, q0=q0, o_f=o_f):
                        nc.sync.dma_start(
                            out=o_d.ap()[b, q0:q0 + QHW, :].rearrange(
                                "(t p) d -> p t d", p=128),
                            in_=o_f)
                    pending.append(store_job)

                while slot < len(next_jobs):
                    next_jobs[slot]()
                    slot += 1

            for _ in range(12):
                warmer()
            jobs0 = prep_load(0)
            for job in jobs0:
                job()
            for b in range(B_SH):
                nxt = prep_load(b + 1) if b + 1 < B_SH else []
                main(b, nxt)
            for job in pending:
                job()

    nc.finalize()
    return nc


def _get_nc():
    if "nc" not in _cached:
        _cached["nc"] = _build()
    return _cached["nc"]


def kernel(query, key, value):
    from concourse.bass_utils import run_bass_kernel_spmd

    nc = _get_nc()
    query = np.ascontiguousarray(query, dtype=np.float32)
    key = np.ascontiguousarray(key, dtype=np.float32)
    value = np.ascontiguousarray(value, dtype=np.float32)

    in_maps = []
    for c in range(NCORES):
        sl = slice(c * B_SH, (c + 1) * B_SH)
        in_maps.append({
            "query": query[sl], "key": key[sl], "value": value[sl]})

    res = run_bass_kernel_spmd(nc, in_maps, core_ids=list(range(NCORES)))
    out = np.concatenate([r["out"] for r in res.results], axis=0)
    return out


# revision 10
# speedup vs baseline: 1.1230x; 1.1029x over previous
"""Dense dot-product attention on 8 Trainium2 NeuronCores.

Problem: query/key/value [32, 2048, 64] fp32 -> softmax(Q K^T / 8) V.
Sharding: batch dim split 4-per-core across 8 cores (data parallel, no
collectives). Each core computes full attention for its 4 batches.

Design (ScalarE-exp-bound, PE kept below the exp floor):
  - All matmuls fp16 (1 cycle/col on the PE; keeps HAM at 2.4 GHz).
  - Q/K transposed to [d, seq] layout by the DMA xbar transpose engine
    (fp16, hidden under compute) -- zero PE transposes. K lands
    pair-packed: khT2[:, p, :] holds K^T tile 2p in partitions 0-63 and
    tile 2p+1 in partitions 64-127, feeding the two 64-row S-matmul
    strips directly. Q is reshuffled into a flat [64, 2048] layout and
    duplicated into both partition halves.
  - S^T[k, q] accumulates into a single persistent PSUM tile
    [128, 3072] = three rotating 1024-col k-tile buffers. exp runs on
    ScalarE over two adjacent buffers at once (N=2048) whenever the
    rotation allows: ~75% of columns go through N=2048 activations,
    cutting the fixed ~350-cycle per-instruction overhead.
  - PV uses the stationary-streaming swap: the exp'd score chunk
    [128k, 128q] is the weight (LoadStationary, FWL at fp16), and
    [V | ones] [128k, 65] streams through. Output accumulates q-MAJOR
    [128q, 65] in PSUM: no output transpose at all, and column 64 is
    the softmax denominator for free. Weight loads double-buffer
    against the streaming matmuls, sustaining ~55-70 ns per 128x128
    weight tile.
  - Normalize: reciprocal of column 64 + per-q-block scalar multiply on
    the DVE, then DMA out.
  - Software pipeline: each plan slot issues S-matmuls for tile group
    i, the exp for group i, then PV matmuls for group i-1, so the PE
    streams while ScalarE exps and vice versa.
"""

import numpy as np

B, L, D = 32, 2048, 64
NCORES = 8
B_SH = B // NCORES          # 4 batches per core
LT = L // 128               # 16 k-tiles of 128
NQH = 2                     # q processed in halves of 1024
QHW = L // NQH              # 1024
NBLK = QHW // 128           # 8 q-blocks of 128 per qh
SCALE = 1.0 / np.sqrt(np.float32(D))  # 0.125

_cached = {}


def _build():
    import concourse.bacc as bacc
    import concourse.tile as tile
    from concourse import mybir

    f32 = mybir.dt.float32
    fp16 = mybir.dt.float16
    Exp = mybir.ActivationFunctionType.Exp

    nc = bacc.Bacc("TRN2", target_bir_lowering=False, debug=False)

    q_d = nc.dram_tensor("query", [B_SH, L, D], f32, kind="ExternalInput")
    k_d = nc.dram_tensor("key", [B_SH, L, D], f32, kind="ExternalInput")
    v_d = nc.dram_tensor("value", [B_SH, L, D], f32, kind="ExternalInput")
    o_d = nc.dram_tensor("out", [B_SH, L, D], f32, kind="ExternalOutput")

    with tile.TileContext(nc) as tc:
        with (
            tc.tile_pool(name="consts", bufs=1) as consts,
            tc.tile_pool(name="nat", bufs=2) as nat,
            tc.tile_pool(name="nath", bufs=2) as nath,
            tc.tile_pool(name="vst", bufs=2) as vst,
            tc.tile_pool(name="qkt", bufs=2) as qkt,
            tc.tile_pool(name="vr", bufs=2) as vrp,
            tc.tile_pool(name="er", bufs=2) as erp,
            tc.tile_pool(name="of32", bufs=2) as of32p,
            tc.tile_pool(name="rz", bufs=8) as rzp,
            tc.tile_pool(name="sps", bufs=3, space="PSUM") as sps,
            tc.tile_pool(name="pvps", bufs=1, space="PSUM") as pvps,
        ):
            # ACT table load + warmers first: they hide under the
            # initial DMA latency and push HAM to 2.4 GHz.
            wsrc = consts.tile([128, 512], fp16)
            nc.vector.memset(wsrc, 1.0)
            dummy = consts.tile([128, 1], f32)
            nc.vector.memset(dummy, 0.0)
            nc.scalar.activation(out=dummy, in_=dummy, func=Exp, scale=1.0)

            warm_ps = pvps.tile([128, NBLK, 128], f32, tag="pv")

            def warmer(n=512):
                nc.tensor.matmul(warm_ps[0:64, 0:n // 128, :], wsrc[:, 0:64],
                                 wsrc[:, 0:n], start=True, stop=True,
                                 skip_group_check=True)

            # per-batch persistent tiles
            tiles = {}  # b -> (qhT, khT2, vr)

            def prep_load(b):
                """DMA loads + fp16 casts + xbar transposes for batch b.

                Returns a list of thunks (woven into the previous
                batch's stream). Dependencies are tracked by Tile, so
                placement only affects issue order.
                """
                q_nat = nat.tile([128, LT, D], f32, tag="qnat")
                k_nat = nat.tile([128, LT, D], f32, tag="knat")
                qh_nat = nath.tile([128, LT, D], fp16, tag="qh_nat")
                kh_nat = nath.tile([128, LT, D], fp16, tag="kh_nat")
                qhT = qkt.tile([128, L], fp16, tag="qhT")
                khT2 = qkt.tile([128, LT // 2, 128], fp16, tag="khT2")
                qstage = qkt.tile([128, LT // 2, 128], fp16, tag="qstage")
                v_stage = vst.tile([128, LT, D], f32, tag="vstage")
                vr = vrp.tile([128, LT, D + 1], fp16, tag="vr")

                q_r = q_d.ap()[b].rearrange("(t p) d -> p t d", p=128)
                k_r = k_d.ap()[b].rearrange("(t p) d -> p t d", p=128)

                jobs = []

                def j(fn):
                    jobs.append(fn)

                # loads (k first: S-matmul weights come from K)
                j(lambda: nc.sync.dma_start(out=k_nat[:, 0:4, :], in_=k_r[:, 0:4, :]))
                j(lambda: nc.sync.dma_start(out=q_nat[:, 0:8, :], in_=q_r[:, 0:8, :]))
                j(lambda: nc.sync.dma_start(out=k_nat[:, 4:LT, :], in_=k_r[:, 4:LT, :]))
                j(lambda: nc.sync.dma_start(out=q_nat[:, 8:LT, :], in_=q_r[:, 8:LT, :]))
                j(lambda: nc.sync.dma_start(
                    out=v_stage, in_=v_d.ap()[b].rearrange("(t p) d -> p t d", p=128)))
                # casts
                j(lambda: nc.vector.tensor_copy(out=kh_nat[:, 0:4, :], in_=k_nat[:, 0:4, :]))
                j(lambda: nc.vector.tensor_copy(out=qh_nat[:, 0:8, :], in_=q_nat[:, 0:8, :]))
                j(lambda: nc.vector.tensor_copy(out=kh_nat[:, 4:LT, :], in_=k_nat[:, 4:LT, :]))
                j(lambda: nc.vector.tensor_copy(out=qh_nat[:, 8:LT, :], in_=q_nat[:, 8:LT, :]))
                # xbar pair-transposes: [128, 2, 64] -> [128, 128] with
                # tile 2p in partitions 0-63, tile 2p+1 in 64-127.
                for p in range(LT // 2):
                    def ktr(p=p):
                        nc.sync.dma_start_transpose(
                            out=khT2[:, p, :], in_=kh_nat[:, 2 * p:2 * p + 2, :])
                    j(ktr)
                for p in range(LT // 2):
                    def qtr(p=p):
                        nc.sync.dma_start_transpose(
                            out=qstage[:, p, :], in_=qh_nat[:, 2 * p:2 * p + 2, :])
                    j(qtr)
                # reshuffle Q^T pairs into flat [64, 2048], then dup to
                # the upper partition half for the strip-b matmuls.
                qhT_t = qhT[0:64, :].rearrange("p (t e c) -> p t e c", e=2, c=128)

                def reshuf_even():
                    nc.sync.dma_start(out=qhT_t[:, :, 0, :], in_=qstage[0:64, :, :])

                def reshuf_odd():
                    nc.sync.dma_start(out=qhT_t[:, :, 1, :], in_=qstage[64:128, :, :])

                j(reshuf_even)
                j(reshuf_odd)
                j(lambda: nc.sync.dma_start(out=qhT[64:128, :], in_=qhT[0:64, :]))
                # V: cast into [V | ones]
                j(lambda: nc.vector.tensor_copy(out=vr[:, :, 0:D], in_=v_stage))
                j(lambda: nc.vector.memset(vr[:, :, D:D + 1], 1.0))

                tiles[b] = (qhT, khT2, vr)
                return jobs

            pending = []   # deferred qh-finish jobs woven into later slots

            state = {"g": 0}  # global k-tile counter (phase for buffers)

            def main(b, next_jobs):
                qhT, khT2, vr = tiles.pop(b)
                slot = 0

                def weave(n):
                    nonlocal slot
                    if pending:
                        pending.pop(0)()
                    for _ in range(n):
                        if slot < len(next_jobs):
                            next_jobs[slot]()
                            slot += 1

                def s_mms(s, t, q0):
                    h = slice(0, 64) if t % 2 == 0 else slice(64, 128)
                    w = khT2[h, t // 2, :]
                    for jj in range(QHW // 512):
                        nc.tensor.matmul(
                            s[:, jj * 512:(jj + 1) * 512],
                            w, qhT[h, q0 + jj * 512:q0 + (jj + 1) * 512],
                            start=True, stop=True)

                def pv_mms(pv, e_of_t):
                    # start=True clears the whole PSUM *bank*, so only
                    # the first block touching each bank (4 blocks/bank
                    # at 512B stride) may carry it; the other blocks'
                    # first write relies on has_written=0 -> overwrite.
                    for t, e_chunk in e_of_t:
                        for blk in range(NBLK):
                            nc.tensor.matmul(
                                pv[:, blk, 0:D + 1],
                                e_chunk[:, blk * 128:(blk + 1) * 128],
                                vr[:, t, :],
                                start=(t == 0 and blk % 4 == 0),
                                stop=(t == LT - 1),
                                skip_group_check=True)

                for qh in range(NQH):
                    q0 = qh * QHW
                    pv = pvps.tile([128, NBLK, 128], f32, tag="pv")
                    prev_pv = None   # (pv, e_of_t) awaiting issue

                    for t in range(LT):
                        weave(2)
                        # own tile object per score buffer: Tile's
                        # object-granular hazard tracking then gives a
                        # 3-deep rotation (write waits on the exp from
                        # 3 tiles ago, which is long done)
                        s = sps.tile([128, QHW], f32, tag="s")
                        s_mms(s, t, q0)
                        e = erp.tile([128, QHW], fp16, tag="e")
                        nc.scalar.activation(
                            out=e, in_=s, func=Exp, scale=float(SCALE))
                        # PV for the previous tile (PE streams under
                        # the current tile's exp)
                        if prev_pv is not None:
                            pv_mms(*prev_pv)
                        prev_pv = (pv, [(t, e)])

                    pv_mms(*prev_pv)
                    state["g"] += LT

                    # deferred finish: normalize q-major [128q, blk, 65]
                    # straight out of PSUM (recip of col 64, then scale)
                    o_f = of32p.tile([128, NBLK, D], f32, tag="of")

                    for blk in range(NBLK):
                        def norm_job(blk=blk, pv=pv, o_f=o_f):
                            rz = rzp.tile([128, 1], f32, tag="rz")
                            nc.vector.reciprocal(
                                out=rz, in_=pv[:, blk, D:D + 1])
                            nc.vector.tensor_scalar_mul(
                                out=o_f[:, blk, :], in0=pv[:, blk, 0:D],
                                scalar1=rz)
                        pending.append(norm_job)

                    def store_job(b=b, q0=q0, o_f=o_f):
                        nc.sync.dma_start(
                            out=o_d.ap()[b, q0:q0 + QHW, :].rearrange(
                                "(t p) d -> p t d", p=128),
                            in_=o_f)
                    pending.append(store_job)

                while slot < len(next_jobs):
                    next_jobs[slot]()
                    slot += 1

            for _ in range(12):
                warmer()
            jobs0 = prep_load(0)
            for job in jobs0:
                job()
            for b in range(B_SH):
                nxt = prep_load(b + 1) if b + 1 < B_SH else []
                main(b, nxt)
            for job in pending:
                job()

    nc.finalize()
    return nc


def _get_nc():
    if "nc" not in _cached:
        _cached["nc"] = _build()
    return _cached["nc"]


def kernel(query, key, value):
    from concourse.bass_utils import run_bass_kernel_spmd

    nc = _get_nc()
    query = np.ascontiguousarray(query, dtype=np.float32)
    key = np.ascontiguousarray(key, dtype=np.float32)
    value = np.ascontiguousarray(value, dtype=np.float32)

    in_maps = []
    for c in range(NCORES):
        sl = slice(c * B_SH, (c + 1) * B_SH)
        in_maps.append({
            "query": query[sl], "key": key[sl], "value": value[sl]})

    res = run_bass_kernel_spmd(nc, in_maps, core_ids=list(range(NCORES)))
    out = np.concatenate([r["out"] for r in res.results], axis=0)
    return out


# revision 11
# speedup vs baseline: 1.2325x; 1.0975x over previous
"""Dense dot-product attention on 8 Trainium2 NeuronCores.

Problem: query/key/value [32, 2048, 64] fp32 -> softmax(Q K^T / 8) V.
Sharding: batch dim split 4-per-core across 8 cores (data parallel, no
collectives). Each core computes full attention for its 4 batches.

Design (ScalarE-exp-bound, PE kept below the exp floor):
  - All matmuls fp16 (1 cycle/col on the PE; keeps HAM at 2.4 GHz).
  - Q/K transposed to [d, seq] layout by the DMA xbar transpose engine
    (fp16, hidden under compute) -- zero PE transposes. K lands
    pair-packed: khT2[:, p, :] holds K^T tile 2p in partitions 0-63 and
    tile 2p+1 in partitions 64-127, feeding the two 64-row S-matmul
    strips directly. Q is reshuffled into a flat [64, 2048] layout and
    duplicated into both partition halves.
  - S^T[k, q] accumulates into a single persistent PSUM tile
    [128, 3072] = three rotating 1024-col k-tile buffers. exp runs on
    ScalarE over two adjacent buffers at once (N=2048) whenever the
    rotation allows: ~75% of columns go through N=2048 activations,
    cutting the fixed ~350-cycle per-instruction overhead.
  - PV uses the stationary-streaming swap: the exp'd score chunk
    [128k, 128q] is the weight (LoadStationary, FWL at fp16), and
    [V | ones] [128k, 65] streams through. Output accumulates q-MAJOR
    [128q, 65] in PSUM: no output transpose at all, and column 64 is
    the softmax denominator for free. Weight loads double-buffer
    against the streaming matmuls, sustaining ~55-70 ns per 128x128
    weight tile.
  - Normalize: reciprocal of column 64 + per-q-block scalar multiply on
    the DVE, then DMA out.
  - Software pipeline: each plan slot issues S-matmuls for tile group
    i, the exp for group i, then PV matmuls for group i-1, so the PE
    streams while ScalarE exps and vice versa.
"""

import numpy as np

B, L, D = 32, 2048, 64
NCORES = 8
B_SH = B // NCORES          # 4 batches per core
LT = L // 128               # 16 k-tiles of 128
NQH = 2                     # q processed in halves of 1024
QHW = L // NQH              # 1024
NBLK = QHW // 128           # 8 q-blocks of 128 per qh
SCALE = 1.0 / np.sqrt(np.float32(D))  # 0.125

_cached = {}


def _build():
    import concourse.bacc as bacc
    import concourse.tile as tile
    from concourse import mybir

    f32 = mybir.dt.float32
    fp16 = mybir.dt.float16
    Exp = mybir.ActivationFunctionType.Exp

    nc = bacc.Bacc("TRN2", target_bir_lowering=False, debug=False)

    q_d = nc.dram_tensor("query", [B_SH, L, D], f32, kind="ExternalInput")
    k_d = nc.dram_tensor("key", [B_SH, L, D], f32, kind="ExternalInput")
    v_d = nc.dram_tensor("value", [B_SH, L, D], f32, kind="ExternalInput")
    o_d = nc.dram_tensor("out", [B_SH, L, D], f32, kind="ExternalOutput")

    with tile.TileContext(nc) as tc:
        with (
            tc.tile_pool(name="consts", bufs=1) as consts,
            tc.tile_pool(name="nat", bufs=2) as nat,
            tc.tile_pool(name="nath", bufs=2) as nath,
            tc.tile_pool(name="vst", bufs=2) as vst,
            tc.tile_pool(name="qkt", bufs=2) as qkt,
            tc.tile_pool(name="vr", bufs=2) as vrp,
            tc.tile_pool(name="er", bufs=2) as erp,
            tc.tile_pool(name="of32", bufs=2) as of32p,
            tc.tile_pool(name="rz", bufs=8) as rzp,
            tc.tile_pool(name="sps", bufs=3, space="PSUM") as sps,
            tc.tile_pool(name="pvps", bufs=1, space="PSUM") as pvps,
        ):
            # ACT table load + warmers first: they hide under the
            # initial DMA latency and push HAM to 2.4 GHz.
            wsrc = consts.tile([128, 512], fp16)
            nc.vector.memset(wsrc, 1.0)
            dummy = consts.tile([128, 1], f32)
            nc.vector.memset(dummy, 0.0)
            nc.scalar.activation(out=dummy, in_=dummy, func=Exp, scale=1.0)

            warm_ps = pvps.tile([128, NBLK, 128], f32, tag="pv")

            def warmer(n=512):
                nc.tensor.matmul(warm_ps[:, 0:n // 128, :], wsrc[:, 0:128],
                                 wsrc[:, 0:n], start=True, stop=True,
                                 skip_group_check=True)

            # per-batch persistent tiles
            tiles = {}  # b -> (qhT, khT2, vr)

            def prep_load(b):
                """DMA loads + fp16 casts + xbar transposes for batch b.

                Returns a list of thunks (woven into the previous
                batch's stream). Dependencies are tracked by Tile, so
                placement only affects issue order.
                """
                q_nat = nat.tile([128, LT, D], f32, tag="qnat")
                k_nat = nat.tile([128, LT, D], f32, tag="knat")
                qh_nat = nath.tile([128, LT, D], fp16, tag="qh_nat")
                kh_nat = nath.tile([128, LT, D], fp16, tag="kh_nat")
                qhT = qkt.tile([128, L], fp16, tag="qhT")
                khT3 = qkt.tile([128, LT, 128], fp16, tag="khT3")
                kstage = qkt.tile([128, LT // 2, 128], fp16, tag="kstage")
                qstage = qkt.tile([128, LT // 2, 128], fp16, tag="qstage")
                v_stage = vst.tile([128, LT, D], f32, tag="vstage")
                vr = vrp.tile([128, LT, D + 1], fp16, tag="vr")

                q_r = q_d.ap()[b].rearrange("(t p) d -> p t d", p=128)
                k_r = k_d.ap()[b].rearrange("(t p) d -> p t d", p=128)

                jobs = []

                def j(fn):
                    jobs.append(fn)

                # loads (k first: S-matmul weights come from K)
                j(lambda: nc.sync.dma_start(out=k_nat[:, 0:4, :], in_=k_r[:, 0:4, :]))
                j(lambda: nc.sync.dma_start(out=q_nat[:, 0:8, :], in_=q_r[:, 0:8, :]))
                j(lambda: nc.sync.dma_start(out=k_nat[:, 4:LT, :], in_=k_r[:, 4:LT, :]))
                j(lambda: nc.sync.dma_start(out=q_nat[:, 8:LT, :], in_=q_r[:, 8:LT, :]))
                j(lambda: nc.sync.dma_start(
                    out=v_stage, in_=v_d.ap()[b].rearrange("(t p) d -> p t d", p=128)))
                # casts
                j(lambda: nc.vector.tensor_copy(out=kh_nat[:, 0:4, :], in_=k_nat[:, 0:4, :]))
                j(lambda: nc.vector.tensor_copy(out=qh_nat[:, 0:8, :], in_=q_nat[:, 0:8, :]))
                j(lambda: nc.vector.tensor_copy(out=kh_nat[:, 4:LT, :], in_=k_nat[:, 4:LT, :]))
                j(lambda: nc.vector.tensor_copy(out=qh_nat[:, 8:LT, :], in_=q_nat[:, 8:LT, :]))
                # xbar pair-transposes: [128, 2, 64] -> [128, 128] with
                # tile 2p in partitions 0-63, tile 2p+1 in 64-127.
                for p in range(LT // 2):
                    def ktr(p=p):
                        nc.sync.dma_start_transpose(
                            out=kstage[:, p, :], in_=kh_nat[:, 2 * p:2 * p + 2, :])
                    j(ktr)
                # reshuffle K^T pairs into per-tile 128-col blocks:
                # even tile t in partitions 0-63 of block t, odd tile in
                # partitions 64-127; the complementary half of each
                # block is ZERO so the S matmul can run with a full
                # C=128 contraction (128x128 tile mode -- the HAM clock
                # monitor ignores row-tiled matmuls, and a 64-row-mode
                # kernel is stuck at 1.2 GHz).
                khT3_v = khT3.rearrange("p (t e) c -> p t e c", e=2)
                if b < 2:
                    # the zero halves are only ever written by these
                    # memsets; buffers rotate 2-deep so batches 2,3
                    # reuse batch 0,1's zeros.
                    j(lambda: nc.vector.memset(khT3_v[64:128, :, 0, :], 0.0))
                    j(lambda: nc.vector.memset(khT3_v[0:64, :, 1, :], 0.0))

                def kshuf_even():
                    nc.sync.dma_start(
                        out=khT3_v[0:64, :, 0, :], in_=kstage[0:64, :, :])

                def kshuf_odd():
                    nc.sync.dma_start(
                        out=khT3_v[64:128, :, 1, :], in_=kstage[64:128, :, :])

                j(kshuf_even)
                j(kshuf_odd)
                for p in range(LT // 2):
                    def qtr(p=p):
                        nc.sync.dma_start_transpose(
                            out=qstage[:, p, :], in_=qh_nat[:, 2 * p:2 * p + 2, :])
                    j(qtr)
                # reshuffle Q^T pairs into flat [64, 2048], then dup to
                # the upper partition half for the strip-b matmuls.
                qhT_t = qhT[0:64, :].rearrange("p (t e c) -> p t e c", e=2, c=128)

                def reshuf_even():
                    nc.sync.dma_start(out=qhT_t[:, :, 0, :], in_=qstage[0:64, :, :])

                def reshuf_odd():
                    nc.sync.dma_start(out=qhT_t[:, :, 1, :], in_=qstage[64:128, :, :])

                j(reshuf_even)
                j(reshuf_odd)
                j(lambda: nc.sync.dma_start(out=qhT[64:128, :], in_=qhT[0:64, :]))
                # V: cast into [V | ones]
                j(lambda: nc.vector.tensor_copy(out=vr[:, :, 0:D], in_=v_stage))
                j(lambda: nc.vector.memset(vr[:, :, D:D + 1], 1.0))

                tiles[b] = (qhT, khT3, vr)
                return jobs

            pending = []   # deferred qh-finish jobs woven into later slots

            state = {"g": 0}  # global k-tile counter (phase for buffers)

            def main(b, next_jobs):
                qhT, khT3, vr = tiles.pop(b)
                slot = 0

                def weave(n):
                    nonlocal slot
                    if pending:
                        pending.pop(0)()
                    for _ in range(n):
                        if slot < len(next_jobs):
                            next_jobs[slot]()
                            slot += 1

                def s_mms(s, t, q0):
                    w = khT3[:, t, :]
                    for jj in range(QHW // 512):
                        nc.tensor.matmul(
                            s[:, jj * 512:(jj + 1) * 512],
                            w, qhT[:, q0 + jj * 512:q0 + (jj + 1) * 512],
                            start=True, stop=True)

                def pv_mms(pv, e_of_t):
                    # start=True clears the whole PSUM *bank*, so only
                    # the first block touching each bank (4 blocks/bank
                    # at 512B stride) may carry it; the other blocks'
                    # first write relies on has_written=0 -> overwrite.
                    for t, e_chunk in e_of_t:
                        for blk in range(NBLK):
                            nc.tensor.matmul(
                                pv[:, blk, 0:D + 1],
                                e_chunk[:, blk * 128:(blk + 1) * 128],
                                vr[:, t, :],
                                start=(t == 0 and blk % 4 == 0),
                                stop=(t == LT - 1),
                                skip_group_check=True)

                for qh in range(NQH):
                    q0 = qh * QHW
                    pv = pvps.tile([128, NBLK, 128], f32, tag="pv")
                    prev_pv = None   # (pv, e_of_t) awaiting issue

                    for t in range(LT):
                        weave(2)
                        # own tile object per score buffer: Tile's
                        # object-granular hazard tracking then gives a
                        # 3-deep rotation (write waits on the exp from
                        # 3 tiles ago, which is long done)
                        s = sps.tile([128, QHW], f32, tag="s")
                        s_mms(s, t, q0)
                        e = erp.tile([128, QHW], fp16, tag="e")
                        nc.scalar.activation(
                            out=e, in_=s, func=Exp, scale=float(SCALE))
                        # PV for the previous tile (PE streams under
                        # the current tile's exp)
                        if prev_pv is not None:
                            pv_mms(*prev_pv)
                        prev_pv = (pv, [(t, e)])

                    pv_mms(*prev_pv)
                    state["g"] += LT

                    # deferred finish: normalize q-major [128q, blk, 65]
                    # straight out of PSUM (recip of col 64, then scale)
                    o_f = of32p.tile([128, NBLK, D], f32, tag="of")

                    for blk in range(NBLK):
                        def norm_job(blk=blk, pv=pv, o_f=o_f):
                            rz = rzp.tile([128, 1], f32, tag="rz")
                            nc.vector.reciprocal(
                                out=rz, in_=pv[:, blk, D:D + 1])
                            nc.vector.tensor_scalar_mul(
                                out=o_f[:, blk, :], in0=pv[:, blk, 0:D],
                                scalar1=rz)
                        pending.append(norm_job)

                    def store_job(b=b, q0=q0, o_f=o_f):
                        nc.sync.dma_start(
                            out=o_d.ap()[b, q0:q0 + QHW, :].rearrange(
                                "(t p) d -> p t d", p=128),
                            in_=o_f)
                    pending.append(store_job)

                while slot < len(next_jobs):
                    next_jobs[slot]()
                    slot += 1

            for _ in range(12):
                warmer()
            jobs0 = prep_load(0)
            for job in jobs0:
                job()
            for b in range(B_SH):
                nxt = prep_load(b + 1) if b + 1 < B_SH else []
                main(b, nxt)
            for job in pending:
                job()

    nc.finalize()
    return nc


def _get_nc():
    if "nc" not in _cached:
        _cached["nc"] = _build()
    return _cached["nc"]


def kernel(query, key, value):
    from concourse.bass_utils import run_bass_kernel_spmd

    nc = _get_nc()
    query = np.ascontiguousarray(query, dtype=np.float32)
    key = np.ascontiguousarray(key, dtype=np.float32)
    value = np.ascontiguousarray(value, dtype=np.float32)

    in_maps = []
    for c in range(NCORES):
        sl = slice(c * B_SH, (c + 1) * B_SH)
        in_maps.append({
            "query": query[sl], "key": key[sl], "value": value[sl]})

    res = run_bass_kernel_spmd(nc, in_maps, core_ids=list(range(NCORES)))
    out = np.concatenate([r["out"] for r in res.results], axis=0)
    return out
